# revision 6
# baseline (speedup 1.0000x reference)
"""Causal self-attention (B=2, T=4096, D=768, H=12) on 8 TRN2 NeuronCores.

Sharding: core c = (batch b = c//4) x (head group g = c%4, 3 heads each).
Each core computes qkv projection for its 3 heads, causal attention, and a
partial output projection (rank-192 slice of W_proj). The host sums the 4
partials per batch and adds b_proj at gather time (the "all-reduce").

Design notes (empirically driven; the kernel is instruction-issue/sem-bound
on this HW, ~170-460ns per instruction, so instruction count and DMA count
matter more than per-matmul FLOPs):
  - All-bf16 matmul datapath (x, W_qkv, Q^T, K^T, V, P, O, W_proj bf16;
    fp32 PSUM accumulate). Measured rel err 3.5e-3 vs the 2e-2 gate.
    bf16 halves all DMA bytes and SBUF footprint.
  - Heads processed SEQUENTIALLY per q-tile: one PV accumulator bank live
    at a time, freeing PSUM for a dedicated projection accumulator (tag
    "pps") so the projection never steals score-ring slots. PSUM: score
    ring 2x[128,1024] + qkv fill + oacc + pps = 8 banks exactly.
  - Q^T/K^T stored per head duplicated across both 64-partition halves;
    score matmuls alternate halves per k-block so consecutive K=64 matmuls
    run in disjoint PE row groups (measured 74ns/matmul vs 520ns same-group).
  - DMAs are BATCHED into single strided descriptors wherever possible
    (x chunk: 1 DMA instead of 6; W_qkv: 1 instead of 6; Q/K partition-half
    duplication: 2 per chunk instead of 6) - each dma_start costs ~2us of
    serialized queue/sem time regardless of size.
  - kb-pair order per head: old pairs ascending, then the diagonal pairs,
    then the newest old pair (hides the Pool mask latency). Causal masking
    uses ONE 2-dim affine_select per diagonal kb-pair (pattern
    [[-128,2],[1,256]] covers both 128-blocks; the widened region is a
    provable no-op where t >= 128 > partition).
  - Scores computed transposed (S^T = K^T^T Q^T per 128-k-block), exp on
    ScalarE (no max-subtraction needed: fp32/bf16 range), causal masking by
    gpsimd affine_select on the exp'd P tiles, PV via V-augmented tiles
    (ones column yields softmax denominators in the same matmul).
"""

import numpy as np

from concourse import bacc, masks, mybir, tile
from concourse.bass_utils import run_bass_kernel_spmd

F32 = mybir.dt.float32
BF16 = mybir.dt.bfloat16
EXP = mybir.ActivationFunctionType.Exp

B, T, D = 2, 4096, 768
H, DK = 12, 64
HPC = 3                  # heads per core
MQ = HPC * DK            # 192 cols per q/k/v slice
MS = 3 * MQ              # 576 total W_qkv slice cols
SCALE = 1.0 / 8.0        # 1/sqrt(DK)

TCH = 512                # token chunk (= q-tile width)
NTCH = T // TCH          # 8
KB = 128                 # k block size
VAW = 3 * (DK + 1)       # 195 cols per k-block in the V-augmented tile

_cached = {}
last_results = None


def _build_nc(repeats=1):
    nc = bacc.Bacc("TRN2", target_bir_lowering=False)

    x_d = nc.dram_tensor("x", [D, T], BF16, kind="ExternalInput")
    wq_d = nc.dram_tensor("wq", [D, MS], BF16, kind="ExternalInput")
    bq_d = nc.dram_tensor("bq", [MS], F32, kind="ExternalInput")
    wp_d = nc.dram_tensor("wp", [MQ, D], BF16, kind="ExternalInput")
    out_d = nc.dram_tensor("out", [T, D], F32, kind="ExternalOutput")

    with tile.TileContext(nc) as tc:
        with (
            tc.tile_pool(name="sbf", bufs=1) as P,
            tc.tile_pool(name="ps", bufs=1, space="PSUM") as PS,
        ):
            for _rep in range(repeats):
                _emit(nc, tc, P, PS, x_d, wq_d, bq_d, wp_d, out_d)

    nc.compile()
    return nc


def _emit(nc, tc, P, PS, x_d, wq_d, bq_d, wp_d, out_d):
    # ---------------- persistent tiles + constant/weight loads ----------------
    ident = P.tile([128, 128], F32, tag="ident")
    masks.make_identity(nc, ident[:])

    w_all = P.tile([128, 6 * MS], BF16, tag="w", name="w_all")
    # interleave weight and round-0 x^T chunk loads (first consumer chases
    # the serial DMA stream)
    xt0 = P.tile([128, 6 * TCH], BF16, tag="xt", bufs=3, name="xt0")
    nc.sync.dma_start(
        w_all[:].rearrange("p (c m) -> p c m", c=6),
        wq_d[:].rearrange("(c p) m -> p c m", c=6),
    )
    nc.sync.dma_start(
        xt0[:].rearrange("p (c t) -> p c t", c=6),
        x_d[:, 0:TCH].rearrange("(c p) t -> p c t", c=6),
    )

    bias_sb = P.tile([128, 5], F32, tag="bias")
    nc.sync.dma_start(
        bias_sb[0:128, 0:4],
        bq_d[0:512].rearrange("(m p) -> p m", m=4),
    )
    nc.sync.dma_start(
        bias_sb[0:64, 4:5], bq_d[512:576].unsqueeze(-1),
    )

    wp0 = P.tile([128, D], BF16, tag="wp0")
    nc.sync.dma_start(wp0[:], wp_d[0:128, :])
    # wp1 duplicated into both PE row-group halves (K=64 proj matmuls for
    # consecutive r-tiles run in disjoint row groups)
    wp1 = P.tile([128, D], BF16, tag="wp1")
    nc.sync.dma_start(wp1[0:64, :], wp_d[128:192, :])
    nc.sync.dma_start(wp1[64:128, :], wp_d[128:192, :])

    # Q^T/K^T per head, bf16, duplicated across partition halves:
    # rows 0:64 and rows 64:128 both hold head h's [64, T] slab (head h at
    # cols h*T). Score matmuls alternate halves per k-block.
    tQK = P.tile([128, 2 * HPC * T], BF16, tag="tqk")
    tQ = tQK[:, 0:HPC * T]
    tK = tQK[:, HPC * T:2 * HPC * T]

    # V augmented, natural layout (bf16): per k-block kb, cols
    # kb*195 + h*65 + (0..63) hold V rows, col kb*195 + h*65 + 64 holds ones.
    vaug = P.tile([128, 32 * VAW], BF16, tag="vaug")
    ones_col = P.tile([128, 1], BF16, tag="ones")
    nc.gpsimd.memset(ones_col[:], 1.0)
    vkb = vaug[:].rearrange("p (kb c) -> p kb c", c=VAW)
    vones = vaug[:].rearrange("p (kb h c) -> p kb h c", h=3, c=65)[:, :, :, 64:65]
    nc.vector.tensor_copy(
        vones, ones_col[:].broadcast_to([128, 96]).rearrange(
            "p (kb h) -> p kb h", h=3).unsqueeze(-1),
    )

    def qk_move(dst, psrc, bias_ap):
        # PSUM -> SBUF with per-partition bias add (+ bf16 cast)
        nc.vector.tensor_scalar_add(dst, psrc, bias_ap)

    # ---- phase 1: x chunk (pre-transposed) -> qkv slices + V blocks ----
    def phase1(t_, xt):
        tcols = slice(t_ * TCH, (t_ + 1) * TCH)
        vst = None
        vst2 = None
        # (dst tile, head, bias column) per 64-row half of each m-group;
        # None = V path
        qk_map = {
            0: ((tQ, 0, 0), (tQ, 1, 0)),
            1: ((tQ, 2, 1), (tK, 0, 1)),
            2: ((tK, 1, 2), (tK, 2, 2)),
        }
        for m in range(5):
            mc = 128 if m < 4 else 64
            acc = PS.tile([128, 512], F32, tag="fill", bufs=1, name="acc")
            for c in range(6):
                nc.tensor.matmul(
                    acc[0:mc, 0:TCH],
                    w_all[:, c * MS + m * 128: c * MS + m * 128 + mc],
                    xt[:, c * TCH:(c + 1) * TCH],
                    start=(c == 0), stop=(c == 5),
                )
            if m < 3:
                for half, (tdst, h, bcol) in enumerate(qk_map[m]):
                    rows = slice(64 * half, 64 * half + 64)
                    dcols = slice(h * T + t_ * TCH, h * T + (t_ + 1) * TCH)
                    qk_move(
                        tdst[0:64, dcols], acc[rows, 0:TCH],
                        bias_sb[rows, m:m + 1],
                    )
            elif m == 3:    # v_h0 | v_h1
                # batched dup: ONE strided DMA covers q+k, all 6 head slabs
                nc.sync.dma_start(
                    tQK[64:128, :].rearrange(
                        "p (h t) -> p h t", h=6
                    )[:, :, t_ * TCH:(t_ + 1) * TCH],
                    tQK[0:64, :].rearrange(
                        "p (h t) -> p h t", h=6
                    )[:, :, t_ * TCH:(t_ + 1) * TCH],
                )
                vst = P.tile([128, TCH], F32, tag="vs", bufs=3, name="vst")
                qk_move(vst[:, :], acc[0:128, 0:TCH], bias_sb[0:128, 3:4])
            else:           # v_h2
                vst2 = P.tile([64, TCH], F32, tag="vs2", bufs=3, name="vst2")
                qk_move(vst2[:, :], acc[0:64, 0:TCH], bias_sb[0:64, 4:5])

        # V^T chunks -> natural-layout V blocks in vaug (f32 transpose via
        # PE as baseline; the vaug copy casts to bf16)
        for r in range(4):
            kb = 4 * t_ + r
            rcols = slice(r * 128, (r + 1) * 128)
            vtp = PS.tile([128, 512], F32, tag="fill", bufs=1, name="vtp")
            nc.tensor.transpose(vtp[0:128, 0:128], vst[:, rcols], ident[:])
            nc.tensor.transpose(
                vtp[0:128, 128:192], vst2[0:64, rcols], ident[0:64, 0:64]
            )
            dst = (
                vaug[:, kb * VAW: kb * VAW + 195]
                .rearrange("p (h c) -> p h c", c=65)[:, :, 0:64]
            )
            src = vtp[:, 0:192].rearrange("p (h c) -> p h c", c=64)
            nc.vector.tensor_copy(dst, src)

    # ---- phase 2: causal attention, heads sequential ----
    def attention(qt, pending_proj):
        npairs = 2 * (qt + 1)
        # pair order: old pairs ascending, then the two diagonal pairs,
        # then the newest old pair last (hides diag mask latency)
        if qt == 0:
            order = [0, 1]
        else:
            order = list(range(npairs - 3)) + [npairs - 2, npairs - 1, npairs - 3]

        ots = []
        for h in range(3):
            oacc = PS.tile([65, TCH], F32, tag="oacc", bufs=1, name=f"oacc{h}")
            first = True
            for oidx, jp in enumerate(order):
                if h == 0 and oidx == 1 and pending_proj is not None:
                    pending_proj()
                    pending_proj = None
                kbs = (2 * jp, 2 * jp + 1)
                trimmed = kbs[0] == 4 * qt + 2
                q0 = 256 if trimmed else 0
                qs = slice(h * T + qt * TCH + q0, h * T + (qt + 1) * TCH)
                last_pair = oidx == len(order) - 1

                s_t = PS.tile([128, 2 * TCH], F32, tag="s", bufs=2, name="s")
                for i, kb in enumerate(kbs):
                    cs = slice(i * TCH + q0, (i + 1) * TCH)
                    half = kb % 2
                    rows = slice(64 * half, 64 * half + 64)
                    krange = slice(h * T + kb * KB, h * T + (kb + 1) * KB)
                    nc.tensor.matmul(
                        s_t[:, cs], tK[rows, krange], tQ[rows, qs],
                        start=True, stop=True,
                    )

                p = P.tile([128, 2 * TCH], BF16, tag="pt", bufs=4, name="p")
                if trimmed:
                    sv = (
                        s_t[:, 256:1024]
                        .rearrange("p (b c) -> p b c", c=256)[:, 0:3:2, :]
                    )
                    pv_dst = (
                        p[:, 256:1024]
                        .rearrange("p (b c) -> p b c", c=256)[:, 0:3:2, :]
                    )
                    nc.scalar.activation(pv_dst, sv, EXP, scale=SCALE)
                else:
                    nc.scalar.activation(p[:], s_t[:], EXP, scale=SCALE)

                if kbs[0] >= 4 * qt:
                    # one affine_select covers both kbs of the diagonal
                    # pair: region cols {off:off+256} U {512+off:512+off+256},
                    # zero where q_local < r*128 + partition. iota =
                    # -128*j + t - p with j the kb-within-pair index; the
                    # extension of r0's 128-wide region to 256 is a no-op
                    # (t >= 128 > p always keeps).
                    off = 256 if trimmed else 0
                    mreg = (
                        p[:].rearrange("pp (j t) -> pp j t", t=TCH)
                        [:, :, off:off + 256]
                    )
                    nc.gpsimd.affine_select(
                        out=mreg, in_=mreg,
                        compare_op=mybir.AluOpType.is_ge,
                        fill=0.0, base=0,
                        pattern=[[-128, 2], [1, 256]], channel_multiplier=-1,
                    )
                for i, kb in enumerate(kbs):
                    pv_off = 0
                    if kb >= 4 * qt:
                        pv_off = (0, 0, 256, 256)[kb - 4 * qt]
                    nc.tensor.matmul(
                        oacc[:, pv_off:TCH],
                        vaug[:, kb * VAW + h * 65: kb * VAW + (h + 1) * 65],
                        p[:, slice(i * TCH + pv_off, (i + 1) * TCH)],
                        start=first,
                        stop=(last_pair and i == 1),
                    )
                    first = False

            # ---- normalize head h: O^T[d,q] * (1/sum[q]) -> bf16 ot ----
            rc = P.tile([1, TCH], F32, tag="rc", bufs=3, name="rc")
            nc.vector.reciprocal(rc[:], oacc[64:65, :])
            rb = P.tile([64, TCH], F32, tag="rb", bufs=3, name="rb")
            nc.gpsimd.partition_broadcast(rb[:], rc[:])
            ots.append((oacc, rb))
            if h == 0:
                ot01 = P.tile([128, TCH], BF16, tag="ot01", bufs=2, name="ot01")
                ot2 = P.tile([128, TCH], BF16, tag="ot2", bufs=2, name="ot2")
            dst = (ot01[0:64, :], ot01[64:128, :], ot2[0:64, :])[h]
            nc.vector.tensor_mul(dst, oacc[0:64, :], rb[:])
            if h == 2:
                nc.vector.tensor_mul(ot2[64:128, :], oacc[0:64, :], rb[:])

        # ---- phase 3: partial projection y = O^T.T @ W_proj_slice ----
        def proj():
            for r in range(4):
                pps = PS.tile([128, D], F32, tag="pps", bufs=1, name="pps")
                tcl = slice(r * 128, (r + 1) * 128)
                rg = slice(0, 64) if r % 2 == 0 else slice(64, 128)
                for ns in (slice(0, 512), slice(512, 768)):
                    nc.tensor.matmul(
                        pps[:, ns], ot01[:, tcl], wp0[:, ns],
                        start=True, stop=False,
                    )
                    nc.tensor.matmul(
                        pps[:, ns], ot2[rg, tcl], wp1[rg, ns],
                        start=False, stop=True,
                    )
                yo = P.tile([128, D], F32, tag="yo", bufs=3, name="yo")
                nc.vector.tensor_copy(yo[:], pps[:, 0:D])
                row0 = qt * TCH + r * 128
                nc.sync.dma_start(out_d[row0:row0 + 128, :], yo[:])

        return proj

    # ---------------- main loop over 512-token rounds ----------------
    phase1(0, xt0)
    pending_proj = None
    for qt in range(NTCH):
        pending_proj = attention(qt, pending_proj)
        if qt + 1 < NTCH:
            xt = P.tile([128, 6 * TCH], BF16, tag="xt", bufs=3, name="xt")
            nc.sync.dma_start(
                xt[:].rearrange("p (c t) -> p c t", c=6),
                x_d[:, (qt + 1) * TCH:(qt + 2) * TCH].rearrange(
                    "(c p) t -> p c t", c=6),
            )
            phase1(qt + 1, xt)
    pending_proj()


def _get_nc():
    if "nc" not in _cached:
        _cached["nc"] = _build_nc()
    return _cached["nc"]


def _make_in_maps(x, W_qkv, b_qkv, W_proj):
    import ml_dtypes
    bf = ml_dtypes.bfloat16
    in_maps = []
    for c in range(8):
        b, g = c // 4, c % 4
        lo, hi = g * MQ, (g + 1) * MQ
        cols = np.r_[lo:hi, D + lo: D + hi, 2 * D + lo: 2 * D + hi]
        in_maps.append({
            "x": np.ascontiguousarray(x[b].T.astype(bf)),
            "wq": np.ascontiguousarray(W_qkv[:, cols].astype(bf)),
            "bq": np.ascontiguousarray(b_qkv[cols]),
            "wp": np.ascontiguousarray(W_proj[lo:hi, :].astype(bf)),
        })
    return in_maps


def kernel(x, W_qkv, b_qkv, W_proj, b_proj):
    global last_results
    x = np.asarray(x, dtype=np.float32)
    W_qkv = np.asarray(W_qkv, dtype=np.float32)
    b_qkv = np.asarray(b_qkv, dtype=np.float32)
    W_proj = np.asarray(W_proj, dtype=np.float32)
    b_proj = np.asarray(b_proj, dtype=np.float32)

    nc = _get_nc()
    in_maps = _make_in_maps(x, W_qkv, b_qkv, W_proj)

    res = run_bass_kernel_spmd(nc, in_maps, core_ids=list(range(8)))
    last_results = res

    y = np.zeros((B, T, D), dtype=np.float32)
    for c in range(8):
        y[c // 4] += res.results[c]["out"]
    y += b_proj[None, None, :]
    return y
